# revision 19
# baseline (speedup 1.0000x reference)
"""GraphSAGE (3x SAGEConv mean-aggr + concat + global_add_pool + linear) on 8 trn2 cores.

Strategy (data-parallel over dst nodes), v4:
- Host: dst-shard nodes 5000/core; per core, permute its nodes by
  -max(half0_deg, half1_deg). Each core ships ONLY its own [SP, F] fp16
  feature slice; an initial AllGather builds the full gather table on-device
  (tables for all 3 layers live in Shared DRAM, rebuilt per layer from the
  computed features via AllGather).
- The gather table is split into two 20032-row halves so gather indices fit
  int16 (dma_gather requirement). Per dst tile of 128 nodes and half h, the
  neighbor lists are packed into K_h[t] slots (uniform across cores, padded
  with a zero-row dummy index).
- ALL device inputs are packed into ONE fp16 blob [BR, 128] per core and
  carved out on-device via AP rearrange+bitcast views; one [G, F] f32 output.
  The per-exec dispatch cost on this runtime is dominated by the NUMBER of
  I/O buffers (~1.8 ms each), not their size.
- Device: per layer, InstDMAGatherAnt TRANSPOSE-mode gathers (feature-major
  output; single_packet must be False in transpose mode or the HW wedges)
  -> DVE per-tile slot-sum directly produces aggT columns -> inv-degree
  scale against a partition-broadcast inv row -> fp16 PE matmuls agg@Wl +
  h@Wr in PSUM -> fused bias+relu on ACT -> PE transpose to node-major ->
  pooling via one-hot matmul into PSUM accumulator + slice write for the
  next layer's AllGather; final pooled partials AllGather + on-chip sum +
  tiny linear, identical on every core.
"""
import os
import shutil
import sys

import numpy as np

sys.path.insert(0, "/opt/trn_rl_repo")

N, E, F, G, C = 40000, 640000, 128, 64, 8
S = N // C            # 5000 real nodes per core
SP = S + 8            # padded per-core slice rows (last 8 rows zero)
NT = 40               # dst tiles of 128 (40*128 = 5120 >= 5000)
P = 128
HALF = C * SP // 2    # 20032 rows per table half
DUMMY = SP - 1        # row 5007 of the first core of each half: zero row
RCAP = int(os.environ.get("BASSK_RCAP", "16"))


def _pack_instrs(K):
    """Greedy-pack tile slot-chunks into gather instructions of <= RCAP slots.

    Entries are (tile, slot_off_in_instr, n_slots, kbase) where kbase is the
    tile's neighbor-slot offset (tiles with K > RCAP span instructions).
    """
    instrs = []
    cur, s = [], 0
    for t in range(NT):
        k = int(K[t])
        kbase = 0
        while k > 0:
            take = min(k, RCAP - s)
            if take == 0:
                instrs.append({"tiles": cur, "ns": s})
                cur, s = [], 0
                continue
            cur.append((t, s, take, kbase))
            s += take
            kbase += take
            k -= take
            if s == RCAP:
                instrs.append({"tiles": cur, "ns": s})
                cur, s = [], 0
    if cur:
        instrs.append({"tiles": cur, "ns": s})
    return instrs


def _host_prep(x, edge_index, batch):
    src, dst = np.asarray(edge_index[0]).astype(np.int64), np.asarray(
        edge_index[1]).astype(np.int64)
    batch = np.asarray(batch)
    deg = np.bincount(dst, minlength=N).astype(np.int64)
    inv = (1.0 / np.maximum(deg, 1)).astype(np.float32)

    d0 = np.zeros(N, np.int64)
    d1 = np.zeros(N, np.int64)
    ehalf0 = (src // S) < (C // 2)
    np.add.at(d0, dst[ehalf0], 1)
    np.add.at(d1, dst[~ehalf0], 1)

    # per-core permutation: sort by -max(d0, d1) (minimizes uniform per-tile
    # max-slot padding for the split gather)
    pg = np.empty(N, dtype=np.int64)               # node -> permuted-global id
    perm_nodes = np.empty((C, S), dtype=np.int64)  # [c, r] -> node
    for c in range(C):
        nodes = np.arange(c * S, (c + 1) * S)
        order = np.argsort(-np.maximum(d0[nodes], d1[nodes]), kind="stable")
        pn = nodes[order]
        perm_nodes[c] = pn
        pg[pn] = c * SP + np.arange(S)

    xp16 = np.zeros((C * SP, F), dtype=np.float16)
    for c in range(C):
        xp16[c * SP: c * SP + S] = x[perm_nodes[c]]

    xsliceT16 = np.zeros((C, F, NT * P), dtype=np.float16)
    for c in range(C):
        xsliceT16[c, :, :S] = xp16[c * SP: c * SP + S].T

    # uniform per-tile slot counts (max over cores)
    K0 = np.zeros(NT, np.int64)
    K1 = np.zeros(NT, np.int64)
    for c in range(C):
        pn = perm_nodes[c]
        for t in range(NT):
            nn = pn[t * P:(t + 1) * P]
            if len(nn) == 0:
                continue
            K0[t] = max(K0[t], d0[nn].max())
            K1[t] = max(K1[t], d1[nn].max())

    instrs = [_pack_instrs(K0), _pack_instrs(K1)]
    # assign idx-tensor column offsets (in units of 16 idxs)
    col = 0
    for h in (0, 1):
        for inst in instrs[h]:
            inst["col16"] = col
            col += inst["ns"] * P // 16
    # pad idx columns to a multiple of 128 so the i16 idx section is a whole
    # number of 256B blob rows per partition
    totc = ((col + 127) // 128) * 128

    # per-core CSR by (half, local dst rank), srcs as half-local permuted ids
    src_pg = pg[src]
    src_half = (src_pg < HALF).astype(np.int64)
    src_local = np.where(src_half == 1, src_pg, src_pg - HALF)
    dst_core = dst // S
    local_rank = np.empty(E, np.int64)
    for c in range(C):
        m = dst_core == c
        local_rank[m] = pg[dst[m]] - c * SP

    idx_arr = np.zeros((C, 128, totc), dtype=np.int16)
    for c in range(C):
        mc = dst_core == c
        for h in (0, 1):
            m = mc & ((src_pg < HALF) if h == 0 else (src_pg >= HALF))
            lr = local_rank[m]
            sp = src_local[m]
            order = np.argsort(lr, kind="stable")
            lr, sp = lr[order], sp[order]
            starts = np.searchsorted(lr, np.arange(S))
            ends = np.searchsorted(lr, np.arange(S) + 1)
            for inst in instrs[h]:
                ns = inst["ns"]
                arr = np.full((ns, P), DUMMY, dtype=np.int16)
                for (t, so, kk, kbase) in inst["tiles"]:
                    r0 = t * P
                    nreal = min(P, S - r0) if r0 < S else 0
                    if nreal <= 0:
                        continue
                    st = starts[r0:r0 + nreal]
                    en = ends[r0:r0 + nreal]
                    for k in range(kk):
                        kg = kbase + k
                        valid = st + kg < en
                        if valid.any():
                            arr[so + k, :nreal][valid] = sp[(st + kg)[valid]]
                flat = arr.reshape(ns * P)
                w = flat.reshape(-1, 16).T          # [16, ni/16]
                w = np.tile(w, (8, 1))              # [128, ni/16]
                cw = inst["col16"]
                idx_arr[c, :, cw:cw + ns * P // 16] = w

    # invT[c, t*P + r] = 1/deg of dst row r of tile t (1.0 for pad rows)
    invT = np.ones((C, NT * P), dtype=np.float32)
    Bmat = np.zeros((C, P, NT, G), dtype=np.float16)
    for c in range(C):
        for t in range(NT):
            r0 = t * P
            nreal = min(P, S - r0) if r0 < S else 0
            if nreal <= 0:
                continue
            nodes = perm_nodes[c][r0:r0 + nreal]
            invT[c, t * P:t * P + nreal] = inv[nodes]
            Bmat[c, np.arange(nreal), t, batch[nodes]] = 1.0
    Bmat = Bmat.reshape(C, P, NT * G)

    plan = {
        "instrs": instrs,
        "totc": totc,
        "K0": tuple(int(v) for v in K0),
        "K1": tuple(int(v) for v in K1),
    }
    return xp16, xsliceT16, idx_arr, invT, Bmat, plan


# ---- blob layout (rows of 128 f16 = 256 B each) ----
def _blob_layout(totc):
    assert (totc * 2) % 256 == 0
    off = {}
    cur = 0
    def sec(name, rows):
        nonlocal cur
        off[name] = cur
        cur += rows
    sec("xslice", SP)                # [SP, F] f16: own permuted slice (AllGather in)
    sec("xsliceT", F * NT * P * 2 // 256)   # [F, NT*P] f16
    sec("idxs", totc * 2 * P // 256)  # [P, totc] i16
    sec("invT", NT * P * 4 // 256)   # [1, NT*P] f32
    sec("Bmat", P * NT * G * 2 // 256)  # [P, NT*G] f16
    sec("W", 6 * P)                  # 6 x [128,128] f16: W1l,W1r,...,W3r
    sec("bias", P * 64 * 4 // 256)   # [P, 64] f32: cols 0-2 = b1..b3, col 3 = blin
    sec("WlinT", P * 3 * F * 4 // 256)  # [F, 3F] f32
    sec("identF", P * P * 4 // 256)  # [128,128] f32
    sec("ident16", P)                # [128,128] f16
    return off, cur


def _build_program(plan):
    import concourse.bass as bass
    import concourse.tile as tile
    from concourse import bacc, library_config, mybir

    f32 = mybir.dt.float32
    f16 = mybir.dt.float16
    i16 = mybir.dt.int16
    totc = plan["totc"]
    instrs = plan["instrs"]
    off, BR = _blob_layout(totc)
    n_queues = int(os.environ.get("BASSK_QUEUES", "2"))
    nc = bacc.Bacc("TRN2", target_bir_lowering=False, debug=False,
                   num_devices=C, num_swdge_queues=n_queues)

    # I/O: ONE blob input, ONE output (per-exec cost is per-buffer)
    blob = nc.dram_tensor("blob", [BR, P], f16, kind="ExternalInput")
    out = nc.dram_tensor("out", [G, F], f32, kind="ExternalOutput")
    dbg = (nc.dram_tensor("dbg", [SP, F], f16, kind="ExternalOutput")
           if os.environ.get("BASSK_DBG") else None)

    def bview(name, rows, bc=None, p=P):
        v = blob[off[name]:off[name] + rows, :].rearrange(
            "(p r) f -> p (r f)", p=p)
        return v.bitcast(bc) if bc is not None else v

    # internals
    slice_x = nc.dram_tensor("slice_x", [SP, F], f16)
    slice_b = [nc.dram_tensor(f"slice_b{l}", [SP, F], f16) for l in (1, 2)]
    hg_space = "Local" if os.environ.get("BASSK_LOCAL_HG") else "Shared"
    hg = [nc.dram_tensor(f"hg{l}", [C * SP, F], f16, addr_space=hg_space)
          for l in (1, 2, 3)]
    pool_b = nc.dram_tensor("pool_b", [F, G], f32)
    pool_r = nc.dram_tensor("pool_r", [F, G], f32, addr_space=hg_space)

    AOT = mybir.AluOpType
    AFT = mybir.ActivationFunctionType
    CH = 512  # matmul free-dim chunk

    no_coll = bool(os.environ.get("BASSK_NO_COLL"))
    no_gather = bool(os.environ.get("BASSK_NO_GATHER"))
    nc.gpsimd.load_library(library_config.mlp)

    with tile.TileContext(nc) as tc:
        with tc.tile_pool(name="persist", bufs=1) as pp, \
             tc.tile_pool(name="g0", bufs=4) as gp0, \
             tc.tile_pool(name="g1", bufs=4) as gp1, \
             tc.tile_pool(name="red", bufs=6) as rp, \
             tc.tile_pool(name="hrow", bufs=3) as hp, \
             tc.tile_pool(name="hT", bufs=2) as hTp, \
             tc.tile_pool(name="agg", bufs=1) as aggp, \
             tc.tile_pool(name="ps_t", bufs=3, space="PSUM") as pst, \
             tc.tile_pool(name="ps_mm", bufs=2, space="PSUM") as psm, \
             tc.tile_pool(name="ps_small", bufs=1, space="PSUM") as pss:

            # build the layer-1 gather table from the sharded slices
            # (collectives cannot read IO tensors: stage via internal DRAM)
            nc.sync.dma_start(out=slice_x[:],
                              in_=blob[off["xslice"]:off["xslice"] + SP, :])
            if no_coll:
                nc.sync.dma_start(out=hg[0][0:SP, :], in_=slice_x[:])
            else:
                nc.gpsimd.collective_compute(
                    "AllGather", AOT.bypass,
                    replica_groups=[list(range(C))],
                    ins=[slice_x[:]],
                    outs=[hg[0][:]],
                )

            identF = pp.tile([P, P], f32, tag="idF")
            nc.sync.dma_start(out=identF[:], in_=bview("identF", 256, f32))
            ident16 = pp.tile([P, P], f16, tag="id16")
            nc.sync.dma_start(out=ident16[:], in_=bview("ident16", P))

            idx_sb = pp.tile([128, totc], i16)
            nc.sync.dma_start(out=idx_sb[:],
                              in_=bview("idxs", totc * 2 * P // 256, i16))
            invT_sb = pp.tile([1, NT * P], f32)
            nc.sync.dma_start(out=invT_sb[:],
                              in_=bview("invT", NT * P * 4 // 256, f32, p=1))
            inv_bc = pp.tile([P, NT * P], f32)
            nc.gpsimd.partition_broadcast(out_ap=inv_bc[:], in_ap=invT_sb[:])
            B_sb = pp.tile([P, NT * G], f16)
            nc.sync.dma_start(out=B_sb[:], in_=bview("Bmat", P * NT * G * 2 // 256))
            bias_sb = pp.tile([P, 64], f32)
            nc.sync.dma_start(out=bias_sb[:], in_=bview("bias", P, f32))
            W_sb = []
            for l in range(3):
                wl = pp.tile([F, F], f16, tag=f"wl{l}")
                nc.sync.dma_start(
                    out=wl[:], in_=blob[off["W"] + 2 * l * P:
                                        off["W"] + (2 * l + 1) * P, :])
                wr = pp.tile([F, F], f16, tag=f"wr{l}")
                nc.sync.dma_start(
                    out=wr[:], in_=blob[off["W"] + (2 * l + 1) * P:
                                        off["W"] + (2 * l + 2) * P, :])
                W_sb.append((wl, bias_sb[:, l:l + 1], wr))
            wlin_sb = pp.tile([F, 3 * F], f32)
            nc.sync.dma_start(out=wlin_sb[:],
                              in_=bview("WlinT", P * 3 * F * 4 // 256, f32))
            blin_ap = bias_sb[:, 3:4]

            zero8 = pp.tile([8, F], f16)
            nc.vector.memset(zero8[:], 0.0)
            for sb in slice_b:
                nc.sync.dma_start(out=sb[S:SP, :], in_=zero8[:])

            hT_cur = pp.tile([F, NT * P], f16, tag="hT0")
            nc.sync.dma_start(out=hT_cur[:],
                              in_=bview("xsliceT", F * NT * P * 2 // 256))

            pooled = pp.tile([G, 3 * F], f32)
            if os.environ.get("BASSK_NO_PE"):
                nc.vector.memset(pooled[:], 0.0)

            n_layers = int(os.environ.get("BASSK_LAYERS", "3"))
            # map tile -> instruction indices per half (tiles may span instrs)
            instr_of_tile = [{}, {}]
            for h in (0, 1):
                for ii, inst in enumerate(instrs[h]):
                    for (t, so, kk, kbase) in inst["tiles"]:
                        instr_of_tile[h].setdefault(t, [])
                        if ii not in instr_of_tile[h][t]:
                            instr_of_tile[h][t].append(ii)

            for l in range(n_layers):
                thalf = [hg[l][0:HALF, :], hg[l][HALF:2 * HALF, :]]
                Wl_sb, b_ap, Wr_sb = W_sb[l]
                aggT = aggp.tile([P, NT * P], f16, tag="aggT")

                issued = [[False] * len(instrs[0]), [False] * len(instrs[1])]
                handles = [{}, {}]
                for t in range(NT):
                    for h in (0, 1):
                        for ii in instr_of_tile[h].get(t, []):
                            if issued[h][ii]:
                                continue
                            inst = instrs[h][ii]
                            ns = inst["ns"]
                            gpool = gp0 if h == 0 else gp1
                            g = gpool.tile([P, RCAP * P], f16, tag=f"g{h}")
                            if no_gather:
                                nc.sync.dma_start(
                                    out=g[:, 0:ns * P],
                                    in_=thalf[h].rearrange(
                                        "(a b) f -> a (b f)", a=P)[:, 0:ns * P])
                            else:
                                # transpose-mode gather: feature-major output.
                                # single_packet=True wedges the HW, and
                                # concurrent transpose gathers on different
                                # SWDGE queues corrupt each other -> one queue.
                                nc.gpsimd.dma_gather(
                                    g[:, 0:ns * P].rearrange(
                                        "p (o k) -> p o k", o=1),
                                    thalf[h],
                                    idx_sb[:, inst["col16"]:
                                           inst["col16"] + ns * P // 16],
                                    ns * P, ns * P, F,
                                    transpose=True,
                                    queue_num=0,
                                    single_packet=False)
                            for (tt, so, kk, kbase) in inst["tiles"]:
                                handles[h].setdefault(tt, []).append((g, so, kk))
                            issued[h][ii] = True

                    dsl = slice(t * P, (t + 1) * P)
                    r_acc = None
                    for h in (0, 1):
                        for (g, so, kk) in handles[h].get(t, []):
                            r = rp.tile([P, P], f32, tag="r")
                            nc.vector.tensor_reduce(
                                out=r[:],
                                in_=g[:, so * P:(so + kk) * P].rearrange(
                                    "p (k r) -> p r k", k=kk),
                                axis=mybir.AxisListType.X, op=AOT.add)
                            if r_acc is None:
                                r_acc = r
                            else:
                                nc.vector.tensor_add(r_acc[:], r_acc[:], r[:])
                    assert r_acc is not None
                    nc.vector.tensor_tensor(
                        out=aggT[:, dsl], in0=r_acc[:], in1=inv_bc[:, dsl],
                        op=AOT.mult)

                if os.environ.get("BASSK_NO_PE"):
                    continue

                # matmuls + bias + relu -> next hT (fp16)
                hT_new = hTp.tile([F, NT * P], f16, tag="hTn")
                for ch in range(NT * P // CH):
                    csl = slice(ch * CH, (ch + 1) * CH)
                    mm = psm.tile([P, CH], f32, tag="mm")
                    nc.tensor.matmul(out=mm[:], lhsT=Wl_sb[:], rhs=aggT[:, csl],
                                     start=True, stop=False)
                    nc.tensor.matmul(out=mm[:], lhsT=Wr_sb[:], rhs=hT_cur[:, csl],
                                     start=False, stop=True)
                    nc.scalar.activation(out=hT_new[:, csl], in_=mm[:],
                                         func=AFT.Relu, bias=b_ap)

                # transpose back per 128-block: pooling matmul + slice write
                pool_ps = pss.tile([G, F], f32, tag="poolps")
                for t in range(NT):
                    tp = pst.tile([P, P], f16, tag="tp16")
                    nc.tensor.transpose(
                        out=tp[:], in_=hT_new[:, t * P:(t + 1) * P],
                        identity=ident16[:])
                    hrow = hp.tile([P, F], f16, tag="hrow")
                    if t % 2 == 0:
                        nc.vector.tensor_copy(out=hrow[:], in_=tp[:])
                    else:
                        nc.scalar.copy(out=hrow[:], in_=tp[:])
                    nc.tensor.matmul(out=pool_ps[:],
                                     lhsT=B_sb[:, t * G:(t + 1) * G],
                                     rhs=hrow[:], start=(t == 0),
                                     stop=(t == NT - 1))
                    if l < 2:
                        r0 = t * P
                        nrows = min(P, S - r0)
                        if nrows > 0:
                            nc.sync.dma_start(out=slice_b[l][r0:r0 + nrows, :],
                                              in_=hrow[:nrows, :])
                nc.vector.tensor_copy(out=pooled[:, l * F:(l + 1) * F],
                                      in_=pool_ps[:])

                if l == 0 and dbg is not None:
                    nc.sync.dma_start(out=dbg[:], in_=slice_b[0][:])
                if l < 2:
                    if no_coll:
                        nc.sync.dma_start(out=hg[l + 1][0:SP, :],
                                          in_=slice_b[l][:])
                    else:
                        nc.gpsimd.collective_compute(
                            "AllGather", AOT.bypass,
                            replica_groups=[list(range(C))],
                            ins=[slice_b[l][:]],
                            outs=[hg[l + 1][:]],
                        )
                hT_cur = hT_new

            # per-core pre-activation final partial: fin = sum_l WlinT_l @
            # pooled_l^T [F, G]; the cross-core sum commutes with the linear,
            # so only a tiny [F, G] AllReduce is needed at the end.
            rhsT = pp.tile([F, 3 * G], f32)
            for l in range(3):
                tp = pss.tile([P, G], f32, tag="tpg")
                nc.tensor.transpose(
                    out=tp[:], in_=pooled[:, l * F:(l + 1) * F],
                    identity=identF[:G, :G])
                nc.vector.tensor_copy(out=rhsT[:, l * G:(l + 1) * G], in_=tp[:])
            fin = pss.tile([F, G], f32, tag="fin")
            for l in range(3):
                nc.tensor.matmul(out=fin[:], lhsT=wlin_sb[:, l * F:(l + 1) * F],
                                 rhs=rhsT[:, l * G:(l + 1) * G],
                                 start=(l == 0), stop=(l == 2))
            fin_sb = pp.tile([F, G], f32, tag="finsb")
            nc.vector.tensor_copy(out=fin_sb[:], in_=fin[:])
            nc.sync.dma_start(out=pool_b[:], in_=fin_sb[:])
            if no_coll:
                nc.sync.dma_start(out=pool_r[:], in_=pool_b[:])
            else:
                nc.gpsimd.collective_compute(
                    "AllReduce", AOT.add, replica_groups=[list(range(C))],
                    ins=[pool_b[:]], outs=[pool_r[:]])
            finR = pp.tile([F, G], f32, tag="finR")
            nc.sync.dma_start(out=finR[:], in_=pool_r[:])
            outT = pp.tile([F, G], f32)
            nc.scalar.activation(out=outT[:], in_=finR[:], func=AFT.Relu,
                                 bias=blin_ap)
            outp = pss.tile([G, F], f32, tag="poolps")
            nc.tensor.transpose(out=outp[:], in_=outT[:], identity=identF[:])
            out_sb = pp.tile([G, F], f32)
            nc.vector.tensor_copy(out=out_sb[:], in_=outp[:])
            nc.sync.dma_start(out=out[:], in_=out_sb[:])

    nc.compile()
    return nc


def _make_runner(nc, n_cores):
    import jax
    from jax.sharding import Mesh, PartitionSpec
    from jax.experimental.shard_map import shard_map
    from concourse import mybir
    from concourse.bass2jax import (_bass_exec_p, install_neuronx_cc_hook,
                                    partition_id_tensor)

    install_neuronx_cc_hook()
    partition_name = nc.partition_id_tensor.name if nc.partition_id_tensor else None
    in_names, out_names, out_avals, zero_outs = [], [], [], []
    for alloc in nc.m.functions[0].allocations:
        if not isinstance(alloc, mybir.MemoryLocationSet):
            continue
        name = alloc.memorylocations[0].name
        if alloc.kind == "ExternalInput":
            if name != partition_name:
                in_names.append(name)
        elif alloc.kind == "ExternalOutput":
            out_names.append(name)
            shape = tuple(alloc.tensor_shape)
            dtype = mybir.dt.np(alloc.dtype)
            out_avals.append(jax.core.ShapedArray(shape, dtype))
            zero_outs.append(np.zeros(shape, dtype))
    n_params = len(in_names)
    in_names_all = in_names + out_names
    if partition_name is not None:
        in_names_all = in_names_all + [partition_name]
    dbg_extra = {}
    if nc.dbg_addr is not None:
        dbg_extra[nc.dbg_addr.name] = np.zeros((1, 2), np.uint32)

    def _body(*args):
        operands = list(args)
        if partition_name is not None:
            operands.append(partition_id_tensor())
        outs = _bass_exec_p.bind(
            *operands, out_avals=tuple(out_avals), in_names=tuple(in_names_all),
            out_names=tuple(out_names), lowering_input_output_aliases=(),
            sim_require_finite=True, sim_require_nnan=True, nc=nc)
        return tuple(outs)

    devices = jax.devices()[:n_cores]
    mesh = Mesh(np.asarray(devices), ("core",))
    nspec = (PartitionSpec("core"),) * (n_params + len(out_avals))
    sharded = jax.jit(
        shard_map(_body, mesh=mesh, in_specs=nspec,
                  out_specs=(PartitionSpec("core"),) * len(out_names),
                  check_rep=False),
        keep_unused=True)

    def run(in_maps):
        per_core = [[np.asarray({**m, **dbg_extra}[name]) for name in in_names]
                    for m in in_maps]
        concat_in = [np.concatenate([per_core[c][i] for c in range(n_cores)], axis=0)
                     for i in range(n_params)]
        concat_zeros = [np.zeros((n_cores * z.shape[0], *z.shape[1:]), z.dtype)
                        for z in zero_outs]
        import jax as _jax
        out_arrs = sharded(*concat_in, *concat_zeros)
        _jax.block_until_ready(out_arrs)
        return [
            {name: np.asarray(out_arrs[i]).reshape(n_cores, *out_avals[i].shape)[c]
             for i, name in enumerate(out_names)}
            for c in range(n_cores)
        ], sharded

    return run


_CACHE = {}


def _make_in_maps(inputs, xp16, xsliceT16, idx_arr, invT, Bmat, plan):
    totc = plan["totc"]
    off, BR = _blob_layout(totc)

    def put(blobc, name, arr):
        b = np.ascontiguousarray(arr).view(np.uint8).reshape(-1, 256)
        r0 = off[name]
        blobc[r0:r0 + b.shape[0]] = b

    Wlin = np.asarray(inputs["Wlin"], dtype=np.float32)
    WlinT = np.ascontiguousarray(
        Wlin.reshape(3, F, F).transpose(1, 0, 2).reshape(F, 3 * F))
    bias_all = np.zeros((P, 64), np.float32)
    for l in (1, 2, 3):
        bias_all[:, l - 1] = np.asarray(inputs[f"b{l}"], np.float32)
    bias_all[:, 3] = np.asarray(inputs["blin"], np.float32)

    common = np.zeros((BR, 256), np.uint8)
    for l in range(3):
        put_off = off["W"] + 2 * l * P
        w = np.asarray(inputs[f"W{l + 1}l"], np.float32).astype(np.float16)
        common[put_off:put_off + P] = np.ascontiguousarray(w).view(
            np.uint8).reshape(-1, 256)
        w = np.asarray(inputs[f"W{l + 1}r"], np.float32).astype(np.float16)
        common[put_off + P:put_off + 2 * P] = np.ascontiguousarray(w).view(
            np.uint8).reshape(-1, 256)
    put(common, "bias", bias_all)
    put(common, "WlinT", WlinT)
    put(common, "identF", np.eye(P, dtype=np.float32))
    put(common, "ident16", np.eye(P, dtype=np.float16))

    in_maps = []
    for c in range(C):
        blobc = common.copy()
        put(blobc, "xslice", xp16[c * SP:(c + 1) * SP])
        put(blobc, "xsliceT", xsliceT16[c])
        put(blobc, "idxs", idx_arr[c])
        put(blobc, "invT", invT[c])
        put(blobc, "Bmat", Bmat[c])
        in_maps.append({"blob": blobc.view(np.float16).reshape(BR, P)})
    return in_maps


def kernel(**inputs):
    x = np.asarray(inputs["x"], dtype=np.float32)
    edge_index = np.asarray(inputs["edge_index"])
    batch = np.asarray(inputs["batch"])

    xp16, xsliceT16, idx_arr, invT, Bmat, plan = _host_prep(
        x, edge_index, batch)

    key = ("prog_v4", plan["K0"], plan["K1"], RCAP,
           os.environ.get("BASSK_LOCAL_HG", ""), os.environ.get("BASSK_QUEUES", ""),
           os.environ.get("BASSK_NO_COLL", ""), os.environ.get("BASSK_NO_GATHER", ""),
           os.environ.get("BASSK_LAYERS", ""), os.environ.get("BASSK_NO_PE", ""),
           os.environ.get("BASSK_DBG", ""))
    if key not in _CACHE:
        shutil.rmtree(os.path.expanduser("~/.neuron-compile-cache"),
                      ignore_errors=True)
        nc = _build_program(plan)
        _CACHE[key] = (_make_runner(nc, C), nc)
    run, nc = _CACHE[key]

    in_maps = _make_in_maps(inputs, xp16, xsliceT16, idx_arr, invT, Bmat, plan)
    results, sharded = run(in_maps)
    kernel._last = (results, plan, in_maps, nc, sharded)
    return results[0]["out"].astype(np.float32)
